# revision 32
# baseline (speedup 1.0000x reference)
"""Trainium2 Bass kernel for nn_MoELayer_67619965108245.

Dense MoE: B=64, N=55, D=512, E=8, L=4 SwiGLU layers per expert, H=2048.
Expert-parallel over 8 NeuronCores (one expert per core).

Layout: all activations live transposed in SBUF as [d_model, tokens]
("dT layout", tokens n-major: t = n*64 + b, N padded 55->56 so T=3584).
This makes every matmul in the SwiGLU chain transpose-free:
  gate^T[h,t] = sum_d Wg[d,h] * normed^T[d,t]      (lhsT = Wg natural)
  delta^T[d,t] = sum_h Wo[h,d] * gv^T[h,t]         (lhsT = Wo natural)
RMSNorm reductions over d (the partition dim) are done with ones-vector
matmuls on the PE; the per-token rstd row is broadcast back across
partitions with a K=1 ones matmul. Squaring runs on GpSimd (SBUF-only
operands) to unload the DVE, which is the bottleneck engine.

The three big matmuls run in fp8-e4m3 with DoubleRow perf mode (2 MACs
per PE cell per cycle, contracting 256 rows per instruction). Scale
management keeps this free of extra ops:
  - the residual stream is carried as h' = HS*h (HS = 4096) in fp32;
    RMSNorm is scale-invariant so `normed` is unchanged and is quantized
    straight to fp8;
  - Wg,Wv are pre-scaled by SWGV=32 on host (fp8 subnormal avoidance);
    silu descales exactly via its input-scale: sil = silu(gps/32);
  - gv is stored as sil*vps = 32*gv_true in fp8 (|32 gv| << 240);
  - Wo is pre-scaled by SWO=128, so the Wo psum is 32*128*delta = HS*delta,
    which is added to h' directly;
  - the router weights Wr and final projection Wp absorb 1/HS on host.

Router: one blocked matmul per 512-token chunk. The 8 nodes of a chunk
are stacked as a [128, 64] block-diagonal stationary operand (rows
8*ni+e), f32r so the PE streams 1 col/cycle; a rank-8 bf16 matmul adds
the row-wise bias AND -1e9 on out-of-block entries so exp() zeroes them;
den/num then reduce over all 64 rows. Router work is emitted interleaved
with the layer-0 rmsnorm pass so it overlaps instead of serializing.

The Wo matmuls + residual adds of chunk c are emitted after the
Wg/Wv/silu/gv loop of chunk c+1 ("one-chunk lag") so the PE starts the
next chunk's gate/val immediately and the DVE never starves at chunk
boundaries (gv pool is double-buffered to make this legal).
"""

import numpy as np
import ml_dtypes

import concourse.bass as bass
import concourse.tile as tile
import concourse.mybir as mybir
from concourse.bass import ds, ts
from concourse.bass_utils import run_bass_kernel_spmd

B, N, D, E, L = 64, 55, 512, 8, 4
H = 4 * D
NP = 56          # padded node count
T = NP * B       # 3584 padded tokens, t = n*B + b
CH = 512         # token chunk (matmul free dim / PSUM bank)
NCH = T // CH    # 7
KD = D // 128    # 4 contraction chunks over d
KH = H // 128    # 16 contraction chunks over h
NPC = CH // B    # nodes per token chunk = 8
EPS = 1e-8
NEG = -1.0e9     # mask for out-of-block router logits

SWGV = 32.0      # host scale on Wg, Wv
SWO = 128.0      # host scale on Wo
HS = SWGV * SWO  # residual-stream scale (4096)

fp32 = mybir.dt.float32
f32r = mybir.dt.float32r
bf16 = mybir.dt.bfloat16
fp8 = mybir.dt.float8e4
bf16_np = ml_dtypes.bfloat16
fp8_np = ml_dtypes.float8_e4m3

DR = mybir.MatmulPerfMode.DoubleRow

# Walrus in this toolchain rejects instructions carrying more than one
# semaphore wait; Tile's final drain aggregates many. Split extras onto
# preceding same-engine NOPs (identical sync semantics).
_MAX_WAITS = 1
SPLIT_WAITS = True  # set False for CoreSim runs (sim rejects no-update NOPs)


def _split_excess_waits(nc, max_waits=_MAX_WAITS):
    if not SPLIT_WAITS:
        return
    for f in nc.m.functions:
        for bb in f.blocks:
            insts = bb.instructions
            i = 0
            while i < len(insts):
                inst = insts[i]
                si = inst.sync_info
                if si is None or si.on_wait is None or len(si.on_wait) <= max_waits:
                    i += 1
                    continue
                waits = list(si.on_wait)
                keep, extra = waits[-max_waits:], waits[:-max_waits]
                nops = []
                for j in range(0, len(extra), max_waits):
                    nops.append(
                        mybir.InstNoOp(
                            name=f"{inst.name}_ws{j}",
                            engine=inst.engine,
                            ins=[],
                            outs=[],
                            sync_info=mybir.SyncInfo(
                                on_wait=extra[j : j + max_waits], on_update=[]
                            ),
                        )
                    )
                inst.sync_info = mybir.SyncInfo(
                    on_wait=keep, on_update=list(si.on_update or [])
                )
                for k, nop in enumerate(nops):
                    insts.insert(i + k, nop)
                i += len(nops) + 1


def build_bass():
    nc = bass.Bass("TRN2", target_bir_lowering=False, debug=False, num_devices=E)

    xT_d = nc.dram_tensor("xT", [KD, 128, T], f32r, kind="ExternalInput").ap()
    wg_d = nc.dram_tensor("wg", [L, 128, KD, H], fp8, kind="ExternalInput").ap()
    wv_d = nc.dram_tensor("wv", [L, 128, KD, H], fp8, kind="ExternalInput").ap()
    wo_d = nc.dram_tensor("wo", [L, 128, KH, D], fp8, kind="ExternalInput").ap()
    wrb_d = nc.dram_tensor("wrb", [128, NCH, KD, NPC * E], f32r, kind="ExternalInput").ap()
    w8_d = nc.dram_tensor("w8b", [NPC, NCH, NPC * E], bf16, kind="ExternalInput").ap()
    v8_d = nc.dram_tensor("v8b", [NPC, CH], bf16, kind="ExternalInput").ap()
    sel_d = nc.dram_tensor("sel64", [NPC * E, 1], bf16, kind="ExternalInput").ap()
    wp_d = nc.dram_tensor("wp", [128, KD, 1], f32r, kind="ExternalInput").ap()
    bp_d = nc.dram_tensor("bps", [1, 1], fp32, kind="ExternalInput").ap()
    u_d = nc.dram_tensor("u", [1, T], fp32, kind="ExternalOutput").ap()

    with tile.TileContext(nc) as tc:
        from contextlib import ExitStack

        with ExitStack() as ctx:
            const = ctx.enter_context(tc.tile_pool(name="const", bufs=1))
            hp = ctx.enter_context(tc.tile_pool(name="hpool", bufs=1))
            wpg = ctx.enter_context(tc.tile_pool(name="wpg", bufs=2))
            wpv = ctx.enter_context(tc.tile_pool(name="wpv", bufs=2))
            wpo = ctx.enter_context(tc.tile_pool(name="wpo", bufs=2))
            nrm = ctx.enter_context(tc.tile_pool(name="nrm", bufs=7))
            sqp = ctx.enter_context(tc.tile_pool(name="sqp", bufs=2))
            bcp = ctx.enter_context(tc.tile_pool(name="bcp", bufs=2))
            gvp = ctx.enter_context(tc.tile_pool(name="gvp", bufs=2))
            silup = ctx.enter_context(tc.tile_pool(name="silup", bufs=3))
            smallp = ctx.enter_context(tc.tile_pool(name="smallp", bufs=4))
            routp = ctx.enter_context(tc.tile_pool(name="routp", bufs=2))
            outp = ctx.enter_context(tc.tile_pool(name="outp", bufs=2))
            pg = ctx.enter_context(tc.tile_pool(name="pg", bufs=3, space="PSUM"))
            pv = ctx.enter_context(tc.tile_pool(name="pv", bufs=2, space="PSUM"))
            pd = ctx.enter_context(tc.tile_pool(name="pd", bufs=2, space="PSUM"))
            pm = ctx.enter_context(tc.tile_pool(name="pm", bufs=1, space="PSUM"))

            # ---- constants ----
            ones_k_bf = const.tile([128, 1], bf16, name="ones_k_bf")
            nc.vector.memset(ones_k_bf, 1.0)
            ones_m_bf = const.tile([1, 128], bf16, name="ones_m_bf")
            nc.vector.memset(ones_m_bf, 1.0)
            ones64 = const.tile([NPC * E, 1], bf16, name="ones64")
            nc.vector.memset(ones64, 1.0)

            eps_sb = const.tile([1, 1], fp32, name="eps_sb")
            nc.vector.memset(eps_sb, EPS)
            zero_sb = const.tile([128, 1], fp32, name="zero_sb")
            nc.vector.memset(zero_sb, 0.0)
            sel_sb = const.tile([NPC * E, 1], bf16, name="sel_sb")
            nc.sync.dma_start(sel_sb[:], sel_d[:])
            w8_sb = const.tile([NPC, NCH, NPC * E], bf16, name="w8_sb")
            nc.sync.dma_start(w8_sb[:], w8_d[:])
            v8_sb = const.tile([NPC, CH], bf16, name="v8_sb")
            nc.sync.dma_start(v8_sb[:], v8_d[:])
            wrb_sb = const.tile([128, NCH, KD, NPC * E], f32r, name="wrb_sb")
            nc.sync.dma_start(wrb_sb[:], wrb_d[:])
            wp_sb = const.tile([128, KD, 1], f32r, name="wp_sb")
            nc.sync.dma_start(wp_sb[:], wp_d[:])
            bp_sb = const.tile([1, 1], fp32, name="bp_sb")
            nc.sync.dma_start(bp_sb[:], bp_d[:])
            w_sb = const.tile([1, T], fp32, name="w_sb")  # router weight row

            # ---- residual state (fp32, dT layout, scaled by HS) ----
            # chunked DMA so chunk-0 work starts before the full load lands
            h = []
            for k in range(KD):
                hk = hp.tile([128, T], f32r, name=f"h{k}", tag=f"h{k}")
                for c in range(NCH):
                    cs = ds(c * CH, CH)
                    nc.sync.dma_start(hk[:, cs], xT_d[k][:, cs])
                h.append(hk)

            NE = NPC * E  # 64 stacked (node, expert) router rows per chunk

            def emit_router_chunk(c):
                cs = ds(c * CH, CH)
                lg = pm.tile([128, CH], fp32, name=f"lg{c}", tag="pm")
                for k in range(KD):
                    nc.tensor.matmul(
                        lg[0:NE, :],
                        wrb_sb[:, c, k, :],
                        h[k][:, cs],
                        start=(k == 0),
                        stop=False,
                    )
                # rank-8 bias: adds br and -1e9 outside each node's column block
                nc.tensor.matmul(
                    lg[0:NE, :],
                    w8_sb[:, c, :],
                    v8_sb[:],
                    start=False,
                    stop=True,
                )
                expc = routp.tile([NE, CH], bf16, name=f"expc{c}", tag="expc")
                with nc.allow_low_precision(reason="softmax exp in bf16"):
                    nc.scalar.activation(
                        expc[:], lg[0:NE, :], mybir.ActivationFunctionType.Exp
                    )
                den = pm.tile([128, CH], fp32, name=f"den{c}", tag="pm")
                nc.tensor.matmul(
                    den[0:1, :], ones64[:], expc[:], start=True, stop=True
                )
                num = pd.tile([128, CH], fp32, name=f"num{c}", tag="pd")
                nc.tensor.matmul(
                    num[0:1, :], sel_sb[:], expc[:], start=True, stop=True
                )
                rden = smallp.tile([1, CH], fp32, name=f"rden{c}", tag="rden")
                nc.scalar.activation(
                    rden[:], den[0:1, :], mybir.ActivationFunctionType.Reciprocal
                )
                nc.vector.tensor_mul(w_sb[:, cs], num[0:1, :], rden[:])

            def emit_rmsnorm_chunk(l, c):
                cs = ds(c * CH, CH)
                sq = sqp.tile([128, KD, CH], bf16, name=f"sq{l}_{c}", tag="sq")
                for k in range(KD):
                    # squaring runs on GpSimd: SBUF-only operands, and it
                    # unloads the DVE (the bottleneck engine)
                    nc.gpsimd.tensor_mul(sq[:, k, :], h[k][:, cs], h[k][:, cs])
                msq = pm.tile([128, CH], fp32, name=f"ms{l}_{c}", tag="pm")
                for k in range(KD):
                    nc.tensor.matmul(
                        msq[0:1, :],
                        ones_k_bf[:],
                        sq[:, k, :],
                        start=(k == 0),
                        stop=(k == KD - 1),
                    )
                # rstd = rsqrt(mean+eps) in ONE ACT op (bf16 is enough: normed
                # is rounded to fp8 right after anyway)
                rstd = smallp.tile([1, CH], bf16, name=f"rstd{l}_{c}", tag="rstd")
                with nc.allow_low_precision(
                    reason="rstd feeds fp8 normed; bf16 rstd is free precision-wise"
                ):
                    nc.scalar.activation(
                        rstd[:],
                        msq[0:1, :],
                        mybir.ActivationFunctionType.Rsqrt,
                        bias=eps_sb[:],
                        scale=1.0 / D,
                    )
                bc = pm.tile([128, CH], fp32, name=f"bc{l}_{c}", tag="pm")
                nc.tensor.matmul(bc[:], ones_m_bf[:], rstd[:], start=True, stop=True)
                # one ACT copy moves bc out of PSUM so the four nt
                # quantize-muls can run on GpSimd (no PSUM port) instead of
                # the bottleneck DVE
                bcs = bcp.tile([128, CH], fp32, name=f"bcs{l}_{c}", tag="bcs")
                nc.scalar.activation(
                    bcs[:], bc[:], mybir.ActivationFunctionType.Identity,
                    bias=zero_sb[:],
                )
                nt = nrm.tile([128, KD, CH], fp8, name=f"nt{l}_{c}", tag="nt")
                for k in range(KD):
                    nc.gpsimd.tensor_mul(nt[:, k, :], h[k][:, cs], bcs[:])
                return nt

            def emit_gv_loop(l, c, nt, wg_sb, wv_sb):
                gv = gvp.tile([128, KH, CH], fp8, name=f"gv{l}_{c}", tag="gv")
                for j in range(KH):
                    gps = pg.tile([128, CH], fp32, name=f"g{l}_{c}_{j}", tag="pg")
                    vps = pv.tile([128, CH], fp32, name=f"v{l}_{c}_{j}", tag="pv")
                    for kk in range(KD // 2):
                        nc.tensor.matmul(
                            gps[:],
                            wg_sb[:, ds(2 * kk, 2), ts(j, 128)],
                            nt[:, ds(2 * kk, 2), :],
                            start=(kk == 0),
                            stop=(kk == KD // 2 - 1),
                            perf_mode=DR,
                        )
                    for kk in range(KD // 2):
                        nc.tensor.matmul(
                            vps[:],
                            wv_sb[:, ds(2 * kk, 2), ts(j, 128)],
                            nt[:, ds(2 * kk, 2), :],
                            start=(kk == 0),
                            stop=(kk == KD // 2 - 1),
                            perf_mode=DR,
                        )
                    sil = silup.tile([128, CH], bf16, name=f"sl{l}_{c}_{j}", tag="sil")
                    nc.scalar.activation(
                        sil[:],
                        gps[:],
                        mybir.ActivationFunctionType.Silu,
                        scale=1.0 / SWGV,
                    )
                    nc.vector.tensor_mul(gv[:, j, :], sil[:], vps[:])
                return gv

            def emit_wo_add(l, c, gv, wo_sb):
                cs = ds(c * CH, CH)
                for i in range(KD):
                    dps = pd.tile([128, CH], fp32, name=f"d{l}_{c}_{i}", tag="pd")
                    for jj in range(KH // 2):
                        nc.tensor.matmul(
                            dps[:],
                            wo_sb[:, ds(2 * jj, 2), ts(i, 128)],
                            gv[:, ds(2 * jj, 2), :],
                            start=(jj == 0),
                            stop=(jj == KH // 2 - 1),
                            perf_mode=DR,
                        )
                    if True:
                        # half the residual adds leave the DVE: ACT drains the
                        # Wo psum to SBUF, GpSimd does the all-SBUF add
                        dsb = bcp.tile([128, CH], fp32, name=f"ds{l}_{c}_{i}", tag="dsb")
                        nc.scalar.activation(
                            dsb[:], dps[:],
                            mybir.ActivationFunctionType.Identity,
                            bias=zero_sb[:],
                        )
                        nc.gpsimd.tensor_add(h[i][:, cs], h[i][:, cs], dsb[:])
                    else:
                        nc.vector.tensor_add(h[i][:, cs], h[i][:, cs], dps[:])

                if l == L - 1:
                    # final projection + router weighting for this chunk
                    eo = pm.tile([128, CH], fp32, name=f"eo{c}", tag="pm")
                    for k in range(KD):
                        nc.tensor.matmul(
                            eo[0:1, :],
                            wp_sb[:, k, :],
                            h[k][:, cs],
                            start=(k == 0),
                            stop=(k == KD - 1),
                        )
                    eos = outp.tile([1, CH], fp32, name=f"eos{c}", tag="eos")
                    nc.scalar.activation(
                        eos[:],
                        eo[0:1, :],
                        mybir.ActivationFunctionType.Identity,
                        bias=bp_sb[:],
                    )
                    us = outp.tile([1, CH], fp32, name=f"us{c}", tag="us")
                    nc.vector.tensor_mul(us[:], eos[:], w_sb[:, cs])
                    nc.sync.dma_start(u_d[0:1, cs], us[:])

            # ---- expert MLP stack (router interleaved into layer 0) ----
            for l in range(L):
                wg_sb = wpg.tile([128, KD, H], fp8, name=f"wg{l}", tag="wg")
                nc.sync.dma_start(wg_sb[:], wg_d[l])
                wv_sb = wpv.tile([128, KD, H], fp8, name=f"wv{l}", tag="wv")
                nc.sync.dma_start(wv_sb[:], wv_d[l])
                wo_sb = wpo.tile([128, KH, D], fp8, name=f"wo{l}", tag="wo")
                nc.sync.dma_start(wo_sb[:], wo_d[l])

                # rmsnorm is emitted staggered two chunks ahead of the
                # SwiGLU loop: ACT is strict FIFO, so front-loading all 7
                # sqrt/bc-copies would stall every layer's first silus behind
                # the whole rmsnorm phase (whose tail depends on the previous
                # layer's last residual adds)
                normed = {}

                def ensure_norm(c):
                    if c < NCH and c not in normed:
                        if l == 0:
                            emit_router_chunk(c)
                        normed[c] = emit_rmsnorm_chunk(l, c)

                ensure_norm(0)
                ensure_norm(1)
                pending = None
                for c in range(NCH):
                    gv = emit_gv_loop(l, c, normed[c], wg_sb, wv_sb)
                    ensure_norm(c + 2)
                    if pending is not None:
                        emit_wo_add(l, pending[0], pending[1], wo_sb)
                    pending = (c, gv)
                emit_wo_add(l, pending[0], pending[1], wo_sb)

    _split_excess_waits(nc)
    return nc


_CACHE = {}


def _get_nc():
    if "nc" not in _CACHE:
        _CACHE["nc"] = build_bass()
    return _CACHE["nc"]


def _fp8(a):
    return np.clip(a, -240.0, 240.0).astype(fp8_np)


def _prep_inputs(x, scale, Wg, Wv, Wo, Wp, bp, Wr, br):
    x = np.asarray(x, np.float32)
    scale = np.asarray(scale, np.float32)
    Wg = np.asarray(Wg, np.float32)
    Wv = np.asarray(Wv, np.float32)
    Wo = np.asarray(Wo, np.float32)
    Wp = np.asarray(Wp, np.float32)
    bp = np.asarray(bp, np.float32)
    Wr = np.asarray(Wr, np.float32)
    br = np.asarray(br, np.float32)

    # xT: [d, n, b] padded -> [KD, 128, T], carried as h' = HS*x
    xt = np.zeros((D, NP, B), np.float32)
    xt[:, :N, :] = x.transpose(2, 1, 0) * HS
    xT = np.ascontiguousarray(xt.reshape(KD, 128, T))

    # blocked router weights (shared by all cores); Wr absorbs 1/HS.
    # wrb[p, c, k, 8*ni+e] = Wr[8c+ni, e, 128k+p] / HS
    wr_full = np.zeros((NP, E, D), np.float32)
    wr_full[:N] = Wr / HS
    wrb = np.ascontiguousarray(
        wr_full.reshape(NCH, NPC, E, KD, 128).transpose(4, 0, 3, 1, 2).reshape(
            128, NCH, KD, NPC * E
        )
    )
    # rank-8 bias: W8[j, c, 8*ni+e] = br[8c+j, e] if ni == j else NEG
    br_full = np.zeros((NP, E), np.float32)
    br_full[:N] = br
    w8 = np.full((NPC, NCH, NPC, E), NEG, np.float32)
    for j in range(NPC):
        w8[j, :, j, :] = br_full.reshape(NCH, NPC, E)[:, j, :]
    w8 = np.ascontiguousarray(w8.reshape(NPC, NCH, NPC * E)).astype(bf16_np)
    v8 = np.zeros((NPC, CH), np.float32)
    for j in range(NPC):
        v8[j, j * B : (j + 1) * B] = 1.0
    v8 = v8.astype(bf16_np)

    # fold RMSNorm scale into Wg/Wv rows: (L, E, D, H)
    wg_eff = Wg * scale[:, :, :, None]
    wv_eff = Wv * scale[:, :, :, None]

    in_maps = []
    for e in range(E):
        wg_p = _fp8(
            np.ascontiguousarray(
                wg_eff[:, e].reshape(L, KD, 128, H).transpose(0, 2, 1, 3)
            )
            * SWGV
        )
        wv_p = _fp8(
            np.ascontiguousarray(
                wv_eff[:, e].reshape(L, KD, 128, H).transpose(0, 2, 1, 3)
            )
            * SWGV
        )
        wo_p = _fp8(
            np.ascontiguousarray(
                Wo[:, e].reshape(L, KH, 128, D).transpose(0, 2, 1, 3)
            )
            * SWO
        )
        wp_p = np.ascontiguousarray(
            Wp[e].reshape(KD, 128, 1).transpose(1, 0, 2) / HS
        )
        sel = np.zeros((NPC * E, 1), np.float32)
        sel[np.arange(NPC) * E + e, 0] = 1.0
        in_maps.append(
            {
                "xT": xT,
                "wg": wg_p,
                "wv": wv_p,
                "wo": wo_p,
                "wrb": wrb,
                "w8b": w8,
                "v8b": v8,
                "sel64": sel.astype(bf16_np),
                "wp": wp_p,
                "bps": np.array([[bp[e]]], np.float32),
            }
        )
    return in_maps


def _combine(results):
    u = np.zeros(T, np.float64)
    for r in results:
        u += r["u"].reshape(T).astype(np.float64)
    return np.ascontiguousarray(u.reshape(NP, B)[:N, :].T).astype(np.float32)


def _healthy(results):
    # a silently-failed core leaves its pre-zeroed output untouched (or
    # NaN/Inf); real per-expert outputs are generically nonzero
    for r in results:
        u = r["u"]
        if not np.isfinite(u).all():
            return False
        if np.abs(u).max() == 0.0:
            return False
    return True


def kernel(x, scale, Wg, Wv, Wo, Wp, bp, Wr, br):
    nc = _get_nc()
    in_maps = _prep_inputs(x, scale, Wg, Wv, Wo, Wp, bp, Wr, br)
    res = run_bass_kernel_spmd(nc, in_maps, list(range(E)))
    for _ in range(2):
        if _healthy(res.results):
            break
        res = run_bass_kernel_spmd(nc, in_maps, list(range(E)))
    return _combine(res.results)
